# revision 21
# baseline (speedup 1.0000x reference)
"""Causal multi-headed self-attention (B=2, S=2048, D=1024, H=16, RoPE)
on 8 Trainium2 NeuronCores.

Sharding: tensor-parallel over heads. Each of the 8 cores owns 2 heads
(a contiguous 128-row slice of wq/wk/wv and 128-column slice of wo) and
computes, for both batches, its heads' projections + RoPE + full causal
attention + its partial contribution to the output projection. The host
sums the 8 partial [B*S, D] outputs.

v2 restructure (vs the 320us baseline):
 - streaming emission: x is loaded in 512-column chunks; projections for
   chunk j and attention block j interleave, so ScalarE exp starts ~5us
   in instead of ~60us.
 - PE warmup matmuls bridge the initial DMA so the HAM clock-gate is at
   8/8 (2.4 GHz) before real work arrives.
 - weights are pre-transposed on the host (no PE transposes, no ident).
 - separate PSUM pools (scores 2x2 banks / AV accum 2 / proj+po 2) so
   cross-phase matmuls can fill PE gaps instead of queueing on one pool.
 - ScalarE runs exp + the ln/exp Z chains (+ a few tail copies); bulk
   copies on DVE, DMA issues on sync/scalar/gpsimd.
 - Z chain split (j=0..2 ahead of block 3, j=3 after) and the zb/norm/po
   work pushed as filler units drained inside attention block 3 and the
   next batch's chunks, so the output projection streams during compute.
 - att-block accumulator copies deferred behind the next chunk's cast so
   the PE's pswap matmul isn't head-of-line blocked on DVE.
 - warm dummy matmuls bridge the startup DMA and the batch-tail latency
   chains so the HAM clock gate stays at 8/8 (2.4 GHz).
 - output staged to [128,1024] bf16 tiles, one DMA per row-tile.
"""

import numpy as np
import ml_dtypes

import bass_rust
from bass_rust import ScopedClock, VectorClock

import concourse.bass as bass
import concourse.mybir as mybir
import concourse.tile as tile
from concourse.bass_utils import run_bass_kernel_spmd

BF = ml_dtypes.bfloat16
F32 = mybir.dt.float32
BF16 = mybir.dt.bfloat16

B, S, D = 2, 2048, 1024
H = 16
DK = 64
ROPE_THETA = 10000.0
NCORES = 8
BS = B * S            # 4096 rows
HD = 2 * DK           # 128: two heads per core
KT = D // 128         # 8 contraction tiles
RT = BS // 128        # 32 row tiles
SQT = S // 512        # 4 query tiles per sequence
SKT = S // 128        # 16 key tiles per sequence
WARMUP = 48           # PE warmup matmuls bridging the initial x DMA


class _TileContext(tile.TileContext):
    """TileContext whose exit drain splits its semaphore waits across
    single-wait NOPs — the walrus build in this environment rejects >1
    sync-wait on TPB_CTRL instructions."""

    def _drain_and_barrier(self, tick_clock, wait_clock):
        n_procs = bass_rust.N_PROCS
        gc = tick_clock.global_clock
        ticks = [gc[p] for p in range(n_procs)]
        for p in range(n_procs):
            if ticks[p] <= 0:
                continue
            sub = VectorClock([ticks[q] if q == p else 0 for q in range(n_procs)])
            nop = self.nc.sync.nop(nofuse=True, hint="drain_wait_split")
            wait_clock.add_sem_waits(nop.ins, ScopedClock({None: sub}))
        self.nc.sync.drain()
        self.nc.all_engine_barrier()
        assert self.sems is not None
        popped = self.nc._tile_sem_poison_stack.pop()
        assert popped is self._sem_poison
        self.nc.clear_and_free_semaphores(list(self.sems.allocated().values()))
        self.nc.all_engine_barrier()


_WSPLIT_CTR = [0]


def _split_multi_waits(nc: bass.Bass, max_waits: int = 1):
    """The walrus build here rejects instructions with more than one
    embedded sync wait. Move extra waits onto same-engine NOP carriers
    emitted immediately before the instruction (program order on the
    engine preserves the semantics)."""
    for f in nc.m.functions:
        for bb in f.blocks:
            insts = bb.instructions
            if not any(
                i.sync_info is not None and len(i.sync_info.on_wait) > max_waits
                for i in insts
            ):
                continue
            new = []
            for inst in insts:
                si = inst.sync_info
                if si is not None and len(si.on_wait) > max_waits:
                    waits = list(si.on_wait)
                    for w in waits[:-max_waits]:
                        _WSPLIT_CTR[0] += 1
                        nop = mybir.InstNoOp(
                            name=f"WSPLIT-{_WSPLIT_CTR[0]}", ins=[], outs=[]
                        )
                        nop.engine = inst.engine
                        nop.sync_info = mybir.SyncInfo(on_wait=[w], on_update=[])
                        new.append(nop)
                    inst.sync_info = mybir.SyncInfo(
                        on_wait=waits[-max_waits:], on_update=list(si.on_update)
                    )
                new.append(inst)
            bb.instructions = new


def build_nc() -> bass.Bass:
    nc = bass.Bass()
    # x stored [KT, 128, BS] (a reshape of xT [D, BS]) so one DMA can
    # move a full 512-column chunk across all 8 kt slices.
    xT = nc.declare_dram_parameter("xT", [KT, 128, BS], BF16, isOutput=False)
    # weights pre-transposed on host: [KT, 128, HD] slices of w.T [D, HD]
    wqT = nc.declare_dram_parameter("wqT", [KT, 128, HD], BF16, isOutput=False)
    wkT = nc.declare_dram_parameter("wkT", [KT, 128, HD], BF16, isOutput=False)
    wvT = nc.declare_dram_parameter("wvT", [KT, 128, HD], BF16, isOutput=False)
    woT = nc.declare_dram_parameter("woT", [HD, D], BF16, isOutput=False)
    cosT = nc.declare_dram_parameter("cosT", [HD, S], BF16, isOutput=False)
    sinT2 = nc.declare_dram_parameter("sinT2", [HD, S], BF16, isOutput=False)
    maskband = nc.declare_dram_parameter("maskband", [128, 128], BF16, isOutput=False)
    pswap = nc.declare_dram_parameter("pswap", [128, 128], BF16, isOutput=False)
    out = nc.declare_dram_parameter("partial", [BS, D], BF16, isOutput=True)

    MULT = mybir.AluOpType.mult
    ADD = mybir.AluOpType.add
    EXP = mybir.ActivationFunctionType.Exp
    LN = mybir.ActivationFunctionType.Ln

    with _TileContext(nc) as tc:
        with (
            tc.tile_pool(name="const", bufs=1) as const,
            tc.tile_pool(name="work", bufs=4) as work,
            tc.tile_pool(name="epool", bufs=6) as epool,
            tc.tile_pool(name="outp", bufs=4) as outp,
            tc.tile_pool(name="scp", bufs=2, space="PSUM") as scp,
            tc.tile_pool(name="accp", bufs=2, space="PSUM") as accp,
            tc.tile_pool(name="pp", bufs=2, space="PSUM") as pp,
            tc.tile_pool(name="ctxup", bufs=1) as ctxup,
        ):
            # ---- persistent SBUF ----
            xT_sb = const.tile([128, KT, BS], BF16)
            wq_sb = const.tile([128, KT, HD], BF16)
            wk_sb = const.tile([128, KT, HD], BF16)
            wv_sb = const.tile([128, KT, HD], BF16)
            wo_sb = const.tile([128, D], BF16)
            cos_sb = const.tile([128, S], BF16)
            sin_sb = const.tile([128, S], BF16)
            mask_sb = const.tile([128, 128], BF16)
            pswap_sb = const.tile([128, 128], BF16)
            qT_sb = const.tile([128, BS], BF16)
            kT_sb = const.tile([128, BS], BF16)
            ctx_sb = const.tile([128, BS], BF16)
            v_sb = const.tile([128, RT, 130], BF16)
            nc.vector.memset(v_sb[:, :, 64:65], 1.0)
            nc.vector.memset(v_sb[:, :, 129:130], 1.0)
            ones_t = const.tile([65, 64], BF16)
            nc.vector.memset(ones_t, 1.0)
            dum_l = const.tile([128, 128], BF16)
            nc.vector.memset(dum_l, 0.0)
            dum_r = const.tile([128, 384], BF16)
            nc.vector.memset(dum_r, 0.0)

            # ---- PE warmup: bridge the initial DMA so HAM hits 8/8 ----
            for wi in range(WARMUP):
                wp = pp.tile([128, 384], F32, tag="proj", name=f"wu{wi}")
                nc.tensor.matmul(wp, lhsT=dum_l, rhs=dum_r, start=True, stop=True)

            # ---- input DMAs (x chunked per 512 cols; weights parallel) ----
            def dma_x_chunk(b, j):
                c0 = b * S + j * 512
                nc.sync.dma_start(
                    out=xT_sb[:, :, c0 : c0 + 512],
                    in_=xT[:, :, c0 : c0 + 512].transpose([1, 0, 2]),
                )

            dma_x_chunk(0, 0)
            for w_dram, w_sb in ((wqT, wq_sb), (wkT, wk_sb), (wvT, wv_sb)):
                nc.scalar.dma_start(out=w_sb, in_=w_dram[:, :, :].transpose([1, 0, 2]))
            nc.gpsimd.dma_start(out=cos_sb, in_=cosT[:, :])
            nc.gpsimd.dma_start(out=sin_sb, in_=sinT2[:, :])
            nc.gpsimd.dma_start(out=mask_sb, in_=maskband[:, :])
            nc.gpsimd.dma_start(out=pswap_sb, in_=pswap[:, :])
            nc.scalar.dma_start(out=wo_sb, in_=woT[:, :])
            for j in range(1, SQT):
                dma_x_chunk(0, j)
            for j in range(SQT):
                dma_x_chunk(1, j)

            # ---- projections for one 512-col chunk ----
            def proj_qk_tile(b, w_sb, dst, jj, nm):
                cs = b * S + jj * 512
                tcs = jj * 512
                ps = pp.tile([128, 512], F32, tag="proj", name=f"pj{nm}{b}{jj}")
                for kt in range(KT):
                    nc.tensor.matmul(
                        ps,
                        lhsT=w_sb[:, kt, :],
                        rhs=xT_sb[:, kt, cs : cs + 512],
                        start=(kt == 0),
                        stop=(kt == KT - 1),
                    )
                qbf = work.tile([128, 512], BF16, tag="qbf")
                nc.vector.tensor_copy(qbf, ps)
                sw = pp.tile([128, 512], F32, tag="proj", name=f"sw{nm}{b}{jj}")
                nc.tensor.matmul(sw, lhsT=pswap_sb, rhs=qbf, start=True, stop=True)
                t1 = work.tile([128, 512], BF16, tag="t1")
                nc.gpsimd.tensor_tensor(t1, qbf, cos_sb[:, tcs : tcs + 512], op=MULT)
                t2 = work.tile([128, 512], BF16, tag="t2")
                nc.vector.tensor_tensor(t2, sw, sin_sb[:, tcs : tcs + 512], op=MULT)
                nc.gpsimd.tensor_tensor(dst[:, cs : cs + 512], t1, t2, op=ADD)

            def proj_v_tile(b, rt):
                grt = b * SKT + rt
                ps = pp.tile([128, 512], F32, tag="proj", name=f"pv{grt}")
                psv = ps[:, 0:128]
                for kt in range(KT):
                    nc.tensor.matmul(
                        psv,
                        lhsT=xT_sb[:, kt, grt * 128 : (grt + 1) * 128],
                        rhs=wv_sb[:, kt, :],
                        start=(kt == 0),
                        stop=(kt == KT - 1),
                    )
                nc.vector.tensor_copy(v_sb[:, grt, 0:64], psv[:, 0:64])
                nc.vector.tensor_copy(v_sb[:, grt, 65:129], psv[:, 64:128])

            filler = []
            _NTW = [0]

            def drain(n):
                for _ in range(min(n, len(filler))):
                    filler.pop(0)()

            def proj_chunk(b, j, att_tail=None):
                proj_qk_tile(b, wq_sb, qT_sb, j, "q")
                # previous att block's accumulator copies go behind the
                # q tile's cast so the PE's sw matmul isn't blocked on DVE
                if att_tail is not None:
                    att_tail()
                drain(1)
                proj_qk_tile(b, wk_sb, kT_sb, j, "k")
                drain(1)
                for rt in range(4 * j, 4 * j + 4):
                    proj_v_tile(b, rt)

            # ---- attention block j for batch b ----
            def att_j(b, j, zrowA, zrowB, ctxu):
                co = b * S
                nsk = 4 * (j + 1)
                acc = [
                    accp.tile([65, 512], F32, tag="acc", name=f"acc{b}{j}{hh}")
                    for hh in range(2)
                ]
                pend = []

                def emit_av(i, c0, e1):
                    for h in range(2):
                        nc.tensor.matmul(
                            acc[h][:, c0:512],
                            lhsT=v_sb[:, b * SKT + i, 65 * h : 65 * h + 65],
                            rhs=e1[:, 512 * h + c0 : 512 * h + 512],
                            start=(i == 0),
                            stop=(i == nsk - 1),
                        )

                for i in range(nsk):
                    sk0 = co + i * 128
                    sq0 = co + j * 512
                    t = 128 * i - 512 * j
                    c0 = max(t, 0)  # causally dead column prefix of this tile
                    ps = scp.tile([128, 1024], F32, tag="sc", name=f"sc{b}{j}{i}")
                    # head 0 narrowed to live columns; head 1 full so the
                    # fused exp below reads only initialized psum
                    nc.tensor.matmul(
                        ps[:, c0:512],
                        lhsT=kT_sb[0:DK, sk0 : sk0 + 128],
                        rhs=qT_sb[0:DK, sq0 + c0 : sq0 + 512],
                        start=True,
                        stop=True,
                    )
                    nc.tensor.matmul(
                        ps[:, 512:1024],
                        lhsT=kT_sb[DK : 2 * DK, sk0 : sk0 + 128],
                        rhs=qT_sb[DK : 2 * DK, sq0 : sq0 + 512],
                        start=True,
                        stop=True,
                    )
                    e1 = epool.tile([128, 1024], BF16, tag="E")
                    nc.scalar.activation(e1[:, c0:1024], ps[:, c0:1024], EXP, scale=0.125)
                    if t >= 0:
                        nc.gpsimd.tensor_tensor(
                            e1[:, c0 : c0 + 128], e1[:, c0 : c0 + 128], mask_sb, op=MULT
                        )
                        nc.gpsimd.tensor_tensor(
                            e1[:, 512 + c0 : 512 + c0 + 128],
                            e1[:, 512 + c0 : 512 + c0 + 128],
                            mask_sb,
                            op=MULT,
                        )
                    pend.append((i, c0, e1))
                    if len(pend) > 2:
                        emit_av(*pend.pop(0))
                    # drain filler at half rate in block 3's first half
                    # (po copies would swamp DVE against the exp pace),
                    # full rate in the second half to shrink the tail
                    if j < 3 or i % 2 == 1:
                        drain(1)
                while pend:
                    emit_av(*pend.pop(0))

                def tail(b=b, j=j, acc=acc):
                    cp = nc.vector.tensor_copy
                    for h in range(2):
                        ct = ctxup.tile(
                            [64, 512], F32, tag=f"ctxu{h}{j}", name=f"ctxu{b}{h}{j}"
                        )
                        ctxu[h][j] = ct
                        cp(ct, acc[h][0:64, :])
                        zdst = (
                            zrowA[32 * j : 32 * j + 1, 512 * h : 512 * h + 512]
                            if j < 3
                            else zrowB[0:1, 512 * h : 512 * h + 512]
                        )
                        cp(zdst, acc[h][64:65, :])

                return tail

            # ---- normalize + output projection, pushed as filler units ----
            def push_norm_j(b, j, ctxu, rz, rz_is_A):
                co = b * S

                def unit(b=b, j=j, ctxu=ctxu, rz=rz, rz_is_A=rz_is_A):
                    for h in range(2):
                        hp = h * DK
                        sq0 = co + j * 512
                        hc = 512 * h
                        rz1 = (
                            rz[32 * j : 32 * j + 1, hc : hc + 512]
                            if rz_is_A
                            else rz[0:1, hc : hc + 512]
                        )
                        on1 = (
                            ones_t[32 * j : 32 * j + 1, :]
                            if rz_is_A
                            else ones_t[0:1, :]
                        )
                        zb = pp.tile([64, 512], F32, tag="proj", name=f"zb{b}{h}{j}")
                        nc.tensor.matmul(zb, lhsT=on1, rhs=rz1, start=True, stop=True)
                        nc.vector.tensor_tensor(
                            ctx_sb[hp : hp + DK, sq0 : sq0 + 512],
                            ctxu[h][j],
                            zb,
                            op=MULT,
                        )

                filler.append(unit)

            def push_po(b, rts):
                for rt in rts:
                    def unit(b=b, rt=rt):
                        grt = b * SKT + rt
                        ot = outp.tile([128, 1024], BF16, tag="o")
                        # batch 0's odd row-tiles borrow the (idle between
                        # batches) AV-accumulator banks so the transition
                        # po stream rotates through 4 banks instead of 2
                        pl, tg = (
                            (accp, "acc") if (b == 0 and rt % 2 == 1) else (pp, "proj")
                        )
                        for od in range(2):
                            ps = pl.tile(
                                [128, 512], F32, tag=tg, name=f"po{grt}{od}"
                            )
                            nc.tensor.matmul(
                                ps,
                                lhsT=ctx_sb[:, grt * 128 : (grt + 1) * 128],
                                rhs=wo_sb[:, od * 512 : (od + 1) * 512],
                                start=True,
                                stop=True,
                            )
                            # the last few units drain at the batch tail
                            # where ScalarE idles and DVE paces the stream
                            if od == 1 and rt >= 10:
                                nc.scalar.copy(ot[:, od * 512 : (od + 1) * 512], ps)
                            else:
                                nc.vector.tensor_copy(
                                    ot[:, od * 512 : (od + 1) * 512], ps
                                )
                        nc.sync.dma_start(
                            out=out[grt * 128 : (grt + 1) * 128, :], in_=ot
                        )

                    filler.append(unit)

            # ---- per-batch streaming schedule ----
            for b in range(2):
                zrowA = ctxup.tile([65, 1024], F32, tag="zrA", name=f"zrA{b}")
                zrowB = ctxup.tile([1, 1024], F32, tag="zrB", name=f"zrB{b}")
                nc.vector.memset(zrowA, 1.0)
                ctxu = [[None] * SQT for _ in range(2)]
                prev_tail = None
                for j in range(SQT - 1):
                    proj_chunk(b, j, att_tail=prev_tail)
                    prev_tail = att_j(b, j, zrowA, zrowB, ctxu)
                proj_chunk(b, 3, att_tail=prev_tail)
                # Z chain for j=0..2 ahead of att block 3; the zb/norm/po
                # units are pushed as filler and drained inside att3's
                # i-loop (and batch b+1's chunks), so the output
                # projection streams during attention
                rzA = ctxup.tile([65, 1024], BF16, tag="rzA", name=f"rzA{b}")
                nc.scalar.activation(zrowA, zrowA, LN)
                nc.scalar.activation(rzA, zrowA, EXP, scale=-1.0)
                for jj_ in (0, 1, 2):
                    push_norm_j(b, jj_, ctxu, rzA, True)
                    push_po(b, range(4 * jj_, 4 * jj_ + 4))
                t3 = att_j(b, 3, zrowA, zrowB, ctxu)
                t3()
                # warm dummies cover the copies -> ln/exp -> zb latency
                # chain so the batch tail doesn't re-throttle the PE
                for _ in range(10):
                    _NTW[0] += 1
                    dp = scp.tile([128, 1024], F32, tag="sc", name=f"bw{_NTW[0]}")
                    nc.tensor.matmul(
                        dp[:, 0:384], lhsT=dum_l, rhs=dum_r, start=True, stop=True
                    )
                rzB = ctxup.tile([1, 1024], BF16, tag="rzB", name=f"rzB{b}")
                nc.scalar.activation(zrowB, zrowB, LN)
                nc.scalar.activation(rzB, zrowB, EXP, scale=-1.0)
                push_norm_j(b, 3, ctxu, rzB, False)
                push_po(b, range(12, 16))
            # tail: keep the PE stream dense with warm dummies between
            # the last few filler units so the clock stays at 8/8
            tw = 0
            while filler:
                drain(1)
                for _ in range(2):
                    tw += 1
                    dp = scp.tile([128, 1024], F32, tag="sc", name=f"tw{tw}")
                    nc.tensor.matmul(
                        dp[:, 0:384], lhsT=dum_l, rhs=dum_r, start=True, stop=True
                    )
    return nc


def host_prep(x, wq, wk, wv, wo, token_positions):
    """Build the per-core input maps (host-side shard + layout prep)."""
    x = np.asarray(x, dtype=np.float32)
    xT = np.ascontiguousarray(x.reshape(BS, D).T).astype(BF)  # [D, BS]
    xT = xT.reshape(KT, 128, BS)

    pos = np.asarray(token_positions).astype(np.float32)  # [S]
    inv = ROPE_THETA ** (-2.0 * np.arange(DK // 2, dtype=np.float32) / DK)
    freqs = np.outer(pos, inv)  # [S, 32]
    cos = np.cos(freqs)
    sin = np.sin(freqs)
    didx = (np.arange(HD) % DK) // 2
    sign = np.where(np.arange(HD) % 2 == 0, -1.0, 1.0).astype(np.float32)
    cosT = np.ascontiguousarray(cos[:, didx].T).astype(np.float32)  # [128, S]
    sinT2 = np.ascontiguousarray(sin[:, didx].T * sign[:, None]).astype(np.float32)

    ii = np.arange(128)[:, None]
    jj = np.arange(128)[None, :]
    maskband = (ii <= jj).astype(np.float32).astype(BF)  # [128, 128]

    pswap = np.zeros((128, 128), np.float32)
    pswap[np.arange(128) ^ 1, np.arange(128)] = 1.0
    pswap = pswap.astype(BF)

    wq = np.asarray(wq, dtype=np.float32)
    wk = np.asarray(wk, dtype=np.float32)
    wv = np.asarray(wv, dtype=np.float32)
    wo = np.asarray(wo, dtype=np.float32)

    in_maps = []
    for c in range(NCORES):
        r0 = c * HD

        def wt(w):
            # [D, HD] pre-transposed slice, tiled [KT, 128, HD]
            return (
                np.ascontiguousarray(w[r0 : r0 + HD, :].T)
                .astype(BF)
                .reshape(KT, 128, HD)
            )

        in_maps.append(
            {
                "xT": xT,
                "wqT": wt(wq),
                "wkT": wt(wk),
                "wvT": wt(wv),
                "woT": np.ascontiguousarray(wo[:, r0 : r0 + HD].T).astype(BF),
                "cosT": cosT.astype(BF),
                "sinT2": sinT2.astype(BF),
                "maskband": maskband,
                "pswap": pswap,
            }
        )
    return in_maps


_NC_CACHE = {}


def get_nc() -> bass.Bass:
    if "nc" not in _NC_CACHE:
        nc = build_nc()
        _split_multi_waits(nc)
        _NC_CACHE["nc"] = nc
    return _NC_CACHE["nc"]


def kernel(x, wq, wk, wv, wo, token_positions, **run_kwargs):
    in_maps = host_prep(x, wq, wk, wv, wo, token_positions)
    nc = get_nc()
    res = run_bass_kernel_spmd(nc, in_maps, list(range(NCORES)), **run_kwargs)
    total = np.zeros((BS, D), np.float64)
    for r in res.results:
        total += r["partial"].astype(np.float64)
    out = total.astype(np.float32).reshape(B, S, D)
    if run_kwargs:
        kernel.last_results = res
    return out


# revision 22
# speedup vs baseline: 1.0041x; 1.0041x over previous
"""Causal multi-headed self-attention (B=2, S=2048, D=1024, H=16, RoPE)
on 8 Trainium2 NeuronCores.

Sharding: tensor-parallel over heads. Each of the 8 cores owns 2 heads
(a contiguous 128-row slice of wq/wk/wv and 128-column slice of wo) and
computes, for both batches, its heads' projections + RoPE + full causal
attention + its partial contribution to the output projection. The host
sums the 8 partial [B*S, D] outputs.

v2 restructure (vs the 320us baseline):
 - streaming emission: x is loaded in 512-column chunks; projections for
   chunk j and attention block j interleave, so ScalarE exp starts ~5us
   in instead of ~60us.
 - PE warmup matmuls bridge the initial DMA so the HAM clock-gate is at
   8/8 (2.4 GHz) before real work arrives.
 - weights are pre-transposed on the host (no PE transposes, no ident).
 - separate PSUM pools (scores 2x2 banks / AV accum 2 / proj+po 2) so
   cross-phase matmuls can fill PE gaps instead of queueing on one pool.
 - ScalarE runs exp + the ln/exp Z chains (+ a few tail copies); bulk
   copies on DVE, DMA issues on sync/scalar/gpsimd.
 - Z chain split (j=0..2 ahead of block 3, j=3 after) and the zb/norm/po
   work pushed as filler units drained inside attention block 3 and the
   next batch's chunks, so the output projection streams during compute.
 - att-block accumulator copies deferred behind the next chunk's cast so
   the PE's pswap matmul isn't head-of-line blocked on DVE.
 - warm dummy matmuls bridge the startup DMA and the batch-tail latency
   chains so the HAM clock gate stays at 8/8 (2.4 GHz).
 - output staged to [128,1024] bf16 tiles, one DMA per row-tile.
"""

import numpy as np
import ml_dtypes

import bass_rust
from bass_rust import ScopedClock, VectorClock

import concourse.bass as bass
import concourse.mybir as mybir
import concourse.tile as tile
from concourse.bass_utils import run_bass_kernel_spmd

BF = ml_dtypes.bfloat16
F32 = mybir.dt.float32
BF16 = mybir.dt.bfloat16

B, S, D = 2, 2048, 1024
H = 16
DK = 64
ROPE_THETA = 10000.0
NCORES = 8
BS = B * S            # 4096 rows
HD = 2 * DK           # 128: two heads per core
KT = D // 128         # 8 contraction tiles
RT = BS // 128        # 32 row tiles
SQT = S // 512        # 4 query tiles per sequence
SKT = S // 128        # 16 key tiles per sequence
WARMUP = 48           # PE warmup matmuls bridging the initial x DMA


class _TileContext(tile.TileContext):
    """TileContext whose exit drain splits its semaphore waits across
    single-wait NOPs — the walrus build in this environment rejects >1
    sync-wait on TPB_CTRL instructions."""

    def _drain_and_barrier(self, tick_clock, wait_clock):
        n_procs = bass_rust.N_PROCS
        gc = tick_clock.global_clock
        ticks = [gc[p] for p in range(n_procs)]
        for p in range(n_procs):
            if ticks[p] <= 0:
                continue
            sub = VectorClock([ticks[q] if q == p else 0 for q in range(n_procs)])
            nop = self.nc.sync.nop(nofuse=True, hint="drain_wait_split")
            wait_clock.add_sem_waits(nop.ins, ScopedClock({None: sub}))
        self.nc.sync.drain()
        self.nc.all_engine_barrier()
        assert self.sems is not None
        popped = self.nc._tile_sem_poison_stack.pop()
        assert popped is self._sem_poison
        self.nc.clear_and_free_semaphores(list(self.sems.allocated().values()))
        self.nc.all_engine_barrier()


_WSPLIT_CTR = [0]


def _split_multi_waits(nc: bass.Bass, max_waits: int = 1):
    """The walrus build here rejects instructions with more than one
    embedded sync wait. Move extra waits onto same-engine NOP carriers
    emitted immediately before the instruction (program order on the
    engine preserves the semantics)."""
    for f in nc.m.functions:
        for bb in f.blocks:
            insts = bb.instructions
            if not any(
                i.sync_info is not None and len(i.sync_info.on_wait) > max_waits
                for i in insts
            ):
                continue
            new = []
            for inst in insts:
                si = inst.sync_info
                if si is not None and len(si.on_wait) > max_waits:
                    waits = list(si.on_wait)
                    for w in waits[:-max_waits]:
                        _WSPLIT_CTR[0] += 1
                        nop = mybir.InstNoOp(
                            name=f"WSPLIT-{_WSPLIT_CTR[0]}", ins=[], outs=[]
                        )
                        nop.engine = inst.engine
                        nop.sync_info = mybir.SyncInfo(on_wait=[w], on_update=[])
                        new.append(nop)
                    inst.sync_info = mybir.SyncInfo(
                        on_wait=waits[-max_waits:], on_update=list(si.on_update)
                    )
                new.append(inst)
            bb.instructions = new


def build_nc() -> bass.Bass:
    nc = bass.Bass()
    # x stored [KT, 128, BS] (a reshape of xT [D, BS]) so one DMA can
    # move a full 512-column chunk across all 8 kt slices.
    xT = nc.declare_dram_parameter("xT", [KT, 128, BS], BF16, isOutput=False)
    # weights pre-transposed on host: [KT, 128, HD] slices of w.T [D, HD]
    wqT = nc.declare_dram_parameter("wqT", [KT, 128, HD], BF16, isOutput=False)
    wkT = nc.declare_dram_parameter("wkT", [KT, 128, HD], BF16, isOutput=False)
    wvT = nc.declare_dram_parameter("wvT", [KT, 128, HD], BF16, isOutput=False)
    woT = nc.declare_dram_parameter("woT", [HD, D], BF16, isOutput=False)
    cosT = nc.declare_dram_parameter("cosT", [HD, S], BF16, isOutput=False)
    sinT2 = nc.declare_dram_parameter("sinT2", [HD, S], BF16, isOutput=False)
    maskband = nc.declare_dram_parameter("maskband", [128, 128], BF16, isOutput=False)
    pswap = nc.declare_dram_parameter("pswap", [128, 128], BF16, isOutput=False)
    out = nc.declare_dram_parameter("partial", [BS, D], BF16, isOutput=True)

    MULT = mybir.AluOpType.mult
    ADD = mybir.AluOpType.add
    EXP = mybir.ActivationFunctionType.Exp
    LN = mybir.ActivationFunctionType.Ln

    with _TileContext(nc) as tc:
        with (
            tc.tile_pool(name="const", bufs=1) as const,
            tc.tile_pool(name="work", bufs=4) as work,
            tc.tile_pool(name="epool", bufs=6) as epool,
            tc.tile_pool(name="outp", bufs=4) as outp,
            tc.tile_pool(name="scp", bufs=2, space="PSUM") as scp,
            tc.tile_pool(name="accp", bufs=2, space="PSUM") as accp,
            tc.tile_pool(name="pp", bufs=2, space="PSUM") as pp,
            tc.tile_pool(name="ctxup", bufs=1) as ctxup,
        ):
            # ---- persistent SBUF ----
            xT_sb = const.tile([128, KT, BS], BF16)
            wq_sb = const.tile([128, KT, HD], BF16)
            wk_sb = const.tile([128, KT, HD], BF16)
            wv_sb = const.tile([128, KT, HD], BF16)
            wo_sb = const.tile([128, D], BF16)
            cos_sb = const.tile([128, S], BF16)
            sin_sb = const.tile([128, S], BF16)
            mask_sb = const.tile([128, 128], BF16)
            pswap_sb = const.tile([128, 128], BF16)
            qT_sb = const.tile([128, BS], BF16)
            kT_sb = const.tile([128, BS], BF16)
            ctx_sb = const.tile([128, BS], BF16)
            v_sb = const.tile([128, RT, 130], BF16)
            nc.vector.memset(v_sb[:, :, 64:65], 1.0)
            nc.vector.memset(v_sb[:, :, 129:130], 1.0)
            ones_t = const.tile([65, 64], BF16)
            nc.vector.memset(ones_t, 1.0)
            dum_l = const.tile([128, 128], BF16)
            nc.vector.memset(dum_l, 0.0)
            dum_r = const.tile([128, 384], BF16)
            nc.vector.memset(dum_r, 0.0)

            # ---- PE warmup: bridge the initial DMA so HAM hits 8/8 ----
            for wi in range(WARMUP):
                wp = pp.tile([128, 384], F32, tag="proj", name=f"wu{wi}")
                nc.tensor.matmul(wp, lhsT=dum_l, rhs=dum_r, start=True, stop=True)

            # ---- input DMAs (x chunked per 512 cols; weights parallel) ----
            def dma_x_chunk(b, j):
                c0 = b * S + j * 512
                nc.sync.dma_start(
                    out=xT_sb[:, :, c0 : c0 + 512],
                    in_=xT[:, :, c0 : c0 + 512].transpose([1, 0, 2]),
                )

            dma_x_chunk(0, 0)
            for w_dram, w_sb in ((wqT, wq_sb), (wkT, wk_sb), (wvT, wv_sb)):
                nc.scalar.dma_start(out=w_sb, in_=w_dram[:, :, :].transpose([1, 0, 2]))
            nc.gpsimd.dma_start(out=cos_sb, in_=cosT[:, :])
            nc.gpsimd.dma_start(out=sin_sb, in_=sinT2[:, :])
            nc.gpsimd.dma_start(out=mask_sb, in_=maskband[:, :])
            nc.gpsimd.dma_start(out=pswap_sb, in_=pswap[:, :])
            nc.scalar.dma_start(out=wo_sb, in_=woT[:, :])
            for j in range(1, SQT):
                dma_x_chunk(0, j)
            for j in range(SQT):
                dma_x_chunk(1, j)

            # ---- projections for one 512-col chunk ----
            def proj_qk_tile(b, w_sb, dst, jj, nm):
                cs = b * S + jj * 512
                tcs = jj * 512
                ps = pp.tile([128, 512], F32, tag="proj", name=f"pj{nm}{b}{jj}")
                for kt in range(KT):
                    nc.tensor.matmul(
                        ps,
                        lhsT=w_sb[:, kt, :],
                        rhs=xT_sb[:, kt, cs : cs + 512],
                        start=(kt == 0),
                        stop=(kt == KT - 1),
                    )
                qbf = work.tile([128, 512], BF16, tag="qbf")
                nc.vector.tensor_copy(qbf, ps)
                sw = pp.tile([128, 512], F32, tag="proj", name=f"sw{nm}{b}{jj}")
                nc.tensor.matmul(sw, lhsT=pswap_sb, rhs=qbf, start=True, stop=True)
                t1 = work.tile([128, 512], BF16, tag="t1")
                nc.gpsimd.tensor_tensor(t1, qbf, cos_sb[:, tcs : tcs + 512], op=MULT)
                t2 = work.tile([128, 512], BF16, tag="t2")
                nc.vector.tensor_tensor(t2, sw, sin_sb[:, tcs : tcs + 512], op=MULT)
                nc.gpsimd.tensor_tensor(dst[:, cs : cs + 512], t1, t2, op=ADD)

            def proj_v_tile(b, rt):
                grt = b * SKT + rt
                ps = pp.tile([128, 512], F32, tag="proj", name=f"pv{grt}")
                psv = ps[:, 0:128]
                for kt in range(KT):
                    nc.tensor.matmul(
                        psv,
                        lhsT=xT_sb[:, kt, grt * 128 : (grt + 1) * 128],
                        rhs=wv_sb[:, kt, :],
                        start=(kt == 0),
                        stop=(kt == KT - 1),
                    )
                nc.vector.tensor_copy(v_sb[:, grt, 0:64], psv[:, 0:64])
                nc.vector.tensor_copy(v_sb[:, grt, 65:129], psv[:, 64:128])

            filler = []
            _NTW = [0]

            def drain(n):
                for _ in range(min(n, len(filler))):
                    filler.pop(0)()

            def proj_chunk(b, j, att_tail=None):
                proj_qk_tile(b, wq_sb, qT_sb, j, "q")
                # previous att block's accumulator copies go behind the
                # q tile's cast so the PE's sw matmul isn't blocked on DVE
                if att_tail is not None:
                    att_tail()
                drain(1)
                proj_qk_tile(b, wk_sb, kT_sb, j, "k")
                drain(1)
                for rt in range(4 * j, 4 * j + 4):
                    proj_v_tile(b, rt)

            # ---- attention block j for batch b ----
            def att_j(b, j, zrowA, zrowB, ctxu):
                co = b * S
                nsk = 4 * (j + 1)
                acc = [
                    accp.tile([65, 512], F32, tag="acc", name=f"acc{b}{j}{hh}")
                    for hh in range(2)
                ]
                pend = []

                def emit_av(i, c0, e1):
                    for h in range(2):
                        nc.tensor.matmul(
                            acc[h][:, c0:512],
                            lhsT=v_sb[:, b * SKT + i, 65 * h : 65 * h + 65],
                            rhs=e1[:, 512 * h + c0 : 512 * h + 512],
                            start=(i == 0),
                            stop=(i == nsk - 1),
                        )

                for i in range(nsk):
                    sk0 = co + i * 128
                    sq0 = co + j * 512
                    t = 128 * i - 512 * j
                    c0 = max(t, 0)  # causally dead column prefix of this tile
                    ps = scp.tile([128, 1024], F32, tag="sc", name=f"sc{b}{j}{i}")
                    # head 0 narrowed to live columns; head 1 full so the
                    # fused exp below reads only initialized psum
                    nc.tensor.matmul(
                        ps[:, c0:512],
                        lhsT=kT_sb[0:DK, sk0 : sk0 + 128],
                        rhs=qT_sb[0:DK, sq0 + c0 : sq0 + 512],
                        start=True,
                        stop=True,
                    )
                    nc.tensor.matmul(
                        ps[:, 512:1024],
                        lhsT=kT_sb[DK : 2 * DK, sk0 : sk0 + 128],
                        rhs=qT_sb[DK : 2 * DK, sq0 : sq0 + 512],
                        start=True,
                        stop=True,
                    )
                    e1 = epool.tile([128, 1024], BF16, tag="E")
                    nc.scalar.activation(e1[:, c0:1024], ps[:, c0:1024], EXP, scale=0.125)
                    if t >= 0:
                        nc.gpsimd.tensor_tensor(
                            e1[:, c0 : c0 + 128], e1[:, c0 : c0 + 128], mask_sb, op=MULT
                        )
                        nc.gpsimd.tensor_tensor(
                            e1[:, 512 + c0 : 512 + c0 + 128],
                            e1[:, 512 + c0 : 512 + c0 + 128],
                            mask_sb,
                            op=MULT,
                        )
                    pend.append((i, c0, e1))
                    if len(pend) > 2:
                        emit_av(*pend.pop(0))
                    # drain filler at half rate in block 3's first half
                    # (po copies would swamp DVE against the exp pace),
                    # full rate in the second half to shrink the tail
                    if j < 3 or i % 2 == 1:
                        drain(1)
                while pend:
                    emit_av(*pend.pop(0))

                def tail(b=b, j=j, acc=acc):
                    cp = nc.vector.tensor_copy
                    for h in range(2):
                        ct = ctxup.tile(
                            [64, 512], F32, tag=f"ctxu{h}{j}", name=f"ctxu{b}{h}{j}"
                        )
                        ctxu[h][j] = ct
                        cp(ct, acc[h][0:64, :])
                        zdst = (
                            zrowA[32 * j : 32 * j + 1, 512 * h : 512 * h + 512]
                            if j < 3
                            else zrowB[0:1, 512 * h : 512 * h + 512]
                        )
                        cp(zdst, acc[h][64:65, :])

                return tail

            # ---- normalize + output projection, pushed as filler units ----
            def push_norm_j(b, j, ctxu, rz, rz_is_A):
                co = b * S

                def unit(b=b, j=j, ctxu=ctxu, rz=rz, rz_is_A=rz_is_A):
                    for h in range(2):
                        hp = h * DK
                        sq0 = co + j * 512
                        hc = 512 * h
                        rz1 = (
                            rz[32 * j : 32 * j + 1, hc : hc + 512]
                            if rz_is_A
                            else rz[0:1, hc : hc + 512]
                        )
                        on1 = (
                            ones_t[32 * j : 32 * j + 1, :]
                            if rz_is_A
                            else ones_t[0:1, :]
                        )
                        zb = pp.tile([64, 512], F32, tag="proj", name=f"zb{b}{h}{j}")
                        nc.tensor.matmul(zb, lhsT=on1, rhs=rz1, start=True, stop=True)
                        nc.vector.tensor_tensor(
                            ctx_sb[hp : hp + DK, sq0 : sq0 + 512],
                            ctxu[h][j],
                            zb,
                            op=MULT,
                        )

                filler.append(unit)

            def push_po(b, rts):
                for rt in rts:
                    def unit(b=b, rt=rt):
                        grt = b * SKT + rt
                        ot = outp.tile([128, 1024], BF16, tag="o")
                        if b == 1 and rt >= 12:
                            # kernel tail: the scores pool is idle, so use
                            # one [128,1024] tile and a single wide copy —
                            # fewer DVE ops on the critical tail chain
                            ps2 = scp.tile([128, 1024], F32, tag="sc", name=f"po{grt}")
                            for od in range(2):
                                nc.tensor.matmul(
                                    ps2[:, od * 512 : (od + 1) * 512],
                                    lhsT=ctx_sb[:, grt * 128 : (grt + 1) * 128],
                                    rhs=wo_sb[:, od * 512 : (od + 1) * 512],
                                    start=True,
                                    stop=True,
                                    skip_group_check=True,
                                )
                            nc.vector.tensor_copy(ot, ps2)
                        else:
                            # batch 0's odd row-tiles borrow the (idle
                            # between batches) AV-accumulator banks so the
                            # transition po stream rotates through 4 banks
                            pl, tg = (
                                (accp, "acc")
                                if (b == 0 and rt % 2 == 1)
                                else (pp, "proj")
                            )
                            for od in range(2):
                                ps = pl.tile(
                                    [128, 512], F32, tag=tg, name=f"po{grt}{od}"
                                )
                                nc.tensor.matmul(
                                    ps,
                                    lhsT=ctx_sb[:, grt * 128 : (grt + 1) * 128],
                                    rhs=wo_sb[:, od * 512 : (od + 1) * 512],
                                    start=True,
                                    stop=True,
                                )
                                # late units drain at the batch tail where
                                # ScalarE idles and DVE paces the stream
                                if od == 1 and rt >= 10:
                                    nc.scalar.copy(
                                        ot[:, od * 512 : (od + 1) * 512], ps
                                    )
                                else:
                                    nc.vector.tensor_copy(
                                        ot[:, od * 512 : (od + 1) * 512], ps
                                    )
                        nc.sync.dma_start(
                            out=out[grt * 128 : (grt + 1) * 128, :], in_=ot
                        )

                    filler.append(unit)

            # ---- per-batch streaming schedule ----
            for b in range(2):
                zrowA = ctxup.tile([65, 1024], F32, tag="zrA", name=f"zrA{b}")
                zrowB = ctxup.tile([1, 1024], F32, tag="zrB", name=f"zrB{b}")
                nc.vector.memset(zrowA, 1.0)
                ctxu = [[None] * SQT for _ in range(2)]
                prev_tail = None
                for j in range(SQT - 1):
                    proj_chunk(b, j, att_tail=prev_tail)
                    prev_tail = att_j(b, j, zrowA, zrowB, ctxu)
                proj_chunk(b, 3, att_tail=prev_tail)
                # Z chain for j=0..2 ahead of att block 3; the zb/norm/po
                # units are pushed as filler and drained inside att3's
                # i-loop (and batch b+1's chunks), so the output
                # projection streams during attention
                rzA = ctxup.tile([65, 1024], BF16, tag="rzA", name=f"rzA{b}")
                nc.scalar.activation(zrowA, zrowA, LN)
                nc.scalar.activation(rzA, zrowA, EXP, scale=-1.0)
                for jj_ in (0, 1, 2):
                    push_norm_j(b, jj_, ctxu, rzA, True)
                    push_po(b, range(4 * jj_, 4 * jj_ + 4))
                t3 = att_j(b, 3, zrowA, zrowB, ctxu)
                t3()
                # warm dummies cover the copies -> ln/exp -> zb latency
                # chain so the batch tail doesn't re-throttle the PE
                for _ in range(10):
                    _NTW[0] += 1
                    dp = scp.tile([128, 1024], F32, tag="sc", name=f"bw{_NTW[0]}")
                    nc.tensor.matmul(
                        dp[:, 0:384], lhsT=dum_l, rhs=dum_r, start=True, stop=True
                    )
                rzB = ctxup.tile([1, 1024], BF16, tag="rzB", name=f"rzB{b}")
                nc.scalar.activation(zrowB, zrowB, LN)
                nc.scalar.activation(rzB, zrowB, EXP, scale=-1.0)
                push_norm_j(b, 3, ctxu, rzB, False)
                push_po(b, range(12, 16))
            # tail: keep the PE stream dense with warm dummies between
            # the last few filler units so the clock stays at 8/8
            tw = 0
            while filler:
                drain(1)
                for _ in range(2):
                    tw += 1
                    dp = scp.tile([128, 1024], F32, tag="sc", name=f"tw{tw}")
                    nc.tensor.matmul(
                        dp[:, 0:384], lhsT=dum_l, rhs=dum_r, start=True, stop=True
                    )
    return nc


def host_prep(x, wq, wk, wv, wo, token_positions):
    """Build the per-core input maps (host-side shard + layout prep)."""
    x = np.asarray(x, dtype=np.float32)
    xT = np.ascontiguousarray(x.reshape(BS, D).T).astype(BF)  # [D, BS]
    xT = xT.reshape(KT, 128, BS)

    pos = np.asarray(token_positions).astype(np.float32)  # [S]
    inv = ROPE_THETA ** (-2.0 * np.arange(DK // 2, dtype=np.float32) / DK)
    freqs = np.outer(pos, inv)  # [S, 32]
    cos = np.cos(freqs)
    sin = np.sin(freqs)
    didx = (np.arange(HD) % DK) // 2
    sign = np.where(np.arange(HD) % 2 == 0, -1.0, 1.0).astype(np.float32)
    cosT = np.ascontiguousarray(cos[:, didx].T).astype(np.float32)  # [128, S]
    sinT2 = np.ascontiguousarray(sin[:, didx].T * sign[:, None]).astype(np.float32)

    ii = np.arange(128)[:, None]
    jj = np.arange(128)[None, :]
    maskband = (ii <= jj).astype(np.float32).astype(BF)  # [128, 128]

    pswap = np.zeros((128, 128), np.float32)
    pswap[np.arange(128) ^ 1, np.arange(128)] = 1.0
    pswap = pswap.astype(BF)

    wq = np.asarray(wq, dtype=np.float32)
    wk = np.asarray(wk, dtype=np.float32)
    wv = np.asarray(wv, dtype=np.float32)
    wo = np.asarray(wo, dtype=np.float32)

    in_maps = []
    for c in range(NCORES):
        r0 = c * HD

        def wt(w):
            # [D, HD] pre-transposed slice, tiled [KT, 128, HD]
            return (
                np.ascontiguousarray(w[r0 : r0 + HD, :].T)
                .astype(BF)
                .reshape(KT, 128, HD)
            )

        in_maps.append(
            {
                "xT": xT,
                "wqT": wt(wq),
                "wkT": wt(wk),
                "wvT": wt(wv),
                "woT": np.ascontiguousarray(wo[:, r0 : r0 + HD].T).astype(BF),
                "cosT": cosT.astype(BF),
                "sinT2": sinT2.astype(BF),
                "maskband": maskband,
                "pswap": pswap,
            }
        )
    return in_maps


_NC_CACHE = {}


def get_nc() -> bass.Bass:
    if "nc" not in _NC_CACHE:
        nc = build_nc()
        _split_multi_waits(nc)
        _NC_CACHE["nc"] = nc
    return _NC_CACHE["nc"]


def kernel(x, wq, wk, wv, wo, token_positions, **run_kwargs):
    in_maps = host_prep(x, wq, wk, wv, wo, token_positions)
    nc = get_nc()
    res = run_bass_kernel_spmd(nc, in_maps, list(range(NCORES)), **run_kwargs)
    total = np.zeros((BS, D), np.float64)
    for r in res.results:
        total += r["partial"].astype(np.float64)
    out = total.astype(np.float32).reshape(B, S, D)
    if run_kwargs:
        kernel.last_results = res
    return out
